# revision 1
# baseline (speedup 1.0000x reference)
"""Trainium2 Bass kernel for nn_LinearRNN (B=16, T=4096, D_in=256, H=512, D_out=256).

  xp = x @ W_in.T                       [B, T, H]
  h_t = xp_t + h_{t-1} @ W_h.T          (W_h is diagonal -> elementwise scan)
  out = hs @ W_out.T                    [B, T, D_out]

Strategy: batch data-parallel over 8 cores (2 batch rows per core). Per core:
  - host pre-transposes x to [b, d, t] so the contraction dim lands on SBUF
    partitions; weights pre-transposed likewise.
  - matmul1 on TensorE produces xp tiles [h=128, t=512] in PSUM,
  - VectorE tensor_tensor_scan runs the recurrence along the free (t) axis
    with the per-h decay broadcast from a [128,1] column, carry chained
    across t-chunks via the previous tile's last column,
  - matmul2 on TensorE contracts h back to d_out, ScalarE copies PSUM->SBUF,
  - output [b, o, t] DMAs back and the host transposes to [b, t, o].
"""
from contextlib import ExitStack

import numpy as np

import concourse.bass as bass
import concourse.mybir as mybir
import concourse.tile as tile
from concourse import bacc
from concourse.bass_utils import run_bass_kernel_spmd

B, T, D_IN, HID, D_OUT = 16, 4096, 256, 512, 256
NCORES = 8
BPC = B // NCORES          # batch rows per core
TC = 512                   # t-chunk (PSUM bank = 512 fp32)
NCH = T // TC
ND = D_IN // 128           # 2  d-blocks
NH = HID // 128            # 4  h-blocks
NO = D_OUT // 128          # 2  o-blocks
OUT_HALF = T // 2

# 'f32'  : exact fp32 matmuls (4 cyc/row on PE)
# 'f32r' : fp32 storage, PE runs reduced-precision single-pass (1 cyc/row)
# 'bf16' : x/weights/hs cast to bf16 (halves input DMA, fastest PE)
MODE_DEFAULT = "f32r"

# schedule/tuning knobs (read by _build; cache key includes them)
CFG = dict(sched="pipe1", xp_bufs=4, op_bufs=4, hs_bufs=16,
           x_piece=512, out_piece=512)

_cache: dict = {}


def _build(mode: str) -> bass.Bass:
    f32 = mybir.dt.float32
    # f32r (tf32): the BIR verifier requires every producer of an fp32r
    # matmul operand to emit fp32r, DMAs included — so the input DRAM params
    # and SBUF tiles carry dt.float32r end-to-end (numpy repr is float32),
    # and the scan writes hs rounded to fp32r.
    dt_in = {"bf16": mybir.dt.bfloat16, "f32r": mybir.dt.float32r}.get(mode, f32)
    dt_hs = dt_in

    def mm(ap):
        return ap

    nc = bacc.Bacc(None, target_bir_lowering=False)

    xT = nc.declare_dram_parameter("xT", [BPC, D_IN, T], dt_in, isOutput=False)
    w_inT = nc.declare_dram_parameter("w_inT", [D_IN, HID], dt_in, isOutput=False)
    w_outT = nc.declare_dram_parameter("w_outT", [HID, D_OUT], dt_in, isOutput=False)
    dcols = nc.declare_dram_parameter("dcols", [128, NH], f32, isOutput=False)
    out = nc.declare_dram_parameter("out", [BPC, D_OUT, T], f32, isOutput=True)

    with tile.TileContext(nc) as tc, ExitStack() as ctx:
        const_pool = ctx.enter_context(tc.tile_pool(name="const", bufs=1))
        x_pool = ctx.enter_context(tc.tile_pool(name="xt", bufs=BPC * ND))
        o_pool = ctx.enter_context(tc.tile_pool(name="ot", bufs=8))
        hs_pool = ctx.enter_context(tc.tile_pool(name="hs", bufs=CFG["hs_bufs"]))
        xp_psum = ctx.enter_context(
            tc.tile_pool(name="xp", bufs=CFG["xp_bufs"], space=bass.MemorySpace.PSUM))
        op_psum = ctx.enter_context(
            tc.tile_pool(name="op", bufs=CFG["op_bufs"], space=bass.MemorySpace.PSUM))

        # DMA emission order is dispatch order per queue: first the matmul1
        # weights, then the first x pieces of batch 0 (unblocks PE ~4 us in),
        # then the remaining constants and the rest of x.
        XP_LEN = CFG["x_piece"]
        xt = {}
        for b in range(BPC):
            for dblk in range(ND):
                xt[(b, dblk)] = x_pool.tile([128, T], dt_in, name="xt", tag="xt")

        def load_x(b, dblk, piece):
            psl = slice(piece * XP_LEN, (piece + 1) * XP_LEN)
            nc.sync.dma_start(xt[(b, dblk)][:, psl],
                              xT[b, dblk * 128:(dblk + 1) * 128, psl])

        for dblk in range(ND):
            load_x(0, dblk, 0)
        wi = []
        for dblk in range(ND):
            w = const_pool.tile([128, HID], dt_in, tag=f"wi{dblk}")
            nc.sync.dma_start(w[:], w_inT[dblk * 128:(dblk + 1) * 128, :])
            wi.append(w)
        wo = []
        for hblk in range(NH):
            w = const_pool.tile([128, D_OUT], dt_in, tag=f"wo{hblk}")
            nc.sync.dma_start(w[:], w_outT[hblk * 128:(hblk + 1) * 128, :])
            wo.append(w)
        dc = const_pool.tile([128, NH], f32, tag="dc")
        nc.sync.dma_start(dc[:], dcols[:])
        for piece in range(1, T // XP_LEN):
            for dblk in range(ND):
                load_x(0, dblk, piece)
        for b in range(1, BPC):
            for piece in range(T // XP_LEN):
                for dblk in range(ND):
                    load_x(b, dblk, piece)

        OP = CFG["out_piece"]
        ot = {}  # (b, oblk) -> current staging piece, created lazily

        prev_hs = {}

        def stage1(b, ic):
            """matmul1 + scan for one (batch, chunk): 4 h-block units."""
            tsl = slice(ic * TC, (ic + 1) * TC)
            for hblk in range(NH):
                xp = xp_psum.tile([128, TC], f32, name="xp", tag="xp")
                for dblk in range(ND):
                    nc.tensor.matmul(
                        xp[:],
                        mm(wi[dblk][:, hblk * 128:(hblk + 1) * 128]),
                        mm(xt[(b, dblk)][:, tsl]),
                        start=(dblk == 0), stop=(dblk == ND - 1))
                hs = hs_pool.tile([128, TC], dt_hs, name="hs", tag="hs")
                init = (0.0 if ic == 0
                        else prev_hs[(b, ic - 1, hblk)][:, TC - 1:TC])
                nc.vector.tensor_tensor_scan(
                    hs[:], dc[:, hblk:hblk + 1].to_broadcast((128, TC)),
                    xp[:], init,
                    op0=mybir.AluOpType.mult, op1=mybir.AluOpType.add)
                prev_hs[(b, ic, hblk)] = hs

        def stage2(b, ic):
            """matmul2 + PSUM->SBUF copy (+ out DMA) for one (batch, chunk)."""
            q, csl = divmod(ic * TC, OP)
            for oblk in range(NO):
                op = op_psum.tile([128, TC], f32, name="op", tag="op")
                for hblk in range(NH):
                    nc.tensor.matmul(
                        op[:],
                        mm(wo[hblk][:, oblk * 128:(oblk + 1) * 128]),
                        mm(prev_hs[(b, ic, hblk)][:]),
                        start=(hblk == 0), stop=(hblk == NH - 1))
                if csl == 0:
                    ot[(b, oblk)] = o_pool.tile([128, OP], f32,
                                                name="ot", tag="ot")
                nc.scalar.copy(ot[(b, oblk)][:, csl:csl + TC], op[:])
                if csl + TC == OP:
                    nc.sync.dma_start(
                        out[b, oblk * 128:(oblk + 1) * 128,
                            q * OP:(q + 1) * OP],
                        ot[(b, oblk)][:])

        sched = CFG.get("sched", "serial")
        if sched == "interleave":
            for ic in range(NCH):
                for b in range(BPC):
                    stage1(b, ic)
                for b in range(BPC):
                    stage2(b, ic)
        elif sched == "pipe1":
            # software pipeline: keep next chunk's matmul1s ahead of the
            # scan-dependent matmul2s in PE program order
            for b in range(BPC):
                stage1(b, 0)
                for ic in range(NCH - 1):
                    stage1(b, ic + 1)
                    stage2(b, ic)
                stage2(b, NCH - 1)
        elif sched == "pipe1x":
            # as pipe1, but cross-batch: b1 chunk 0 primes before b0 drains
            order = [(b, ic) for b in range(BPC) for ic in range(NCH)]
            stage1(*order[0])
            for k in range(len(order) - 1):
                stage1(*order[k + 1])
                stage2(*order[k])
            stage2(*order[-1])
        else:
            for b in range(BPC):
                for ic in range(NCH):
                    stage1(b, ic)
                    stage2(b, ic)

    nc.compile()
    return nc


def _prep_inputs(x, W_in, W_h, W_out, mode: str):
    npdt = np.float32
    if mode == "bf16":
        import ml_dtypes
        npdt = ml_dtypes.bfloat16
    xT = np.ascontiguousarray(np.transpose(np.asarray(x, np.float32), (0, 2, 1))).astype(npdt)
    w_inT = np.ascontiguousarray(np.asarray(W_in, np.float32).T).astype(npdt)
    w_outT = np.ascontiguousarray(np.asarray(W_out, np.float32).T).astype(npdt)
    d = np.ascontiguousarray(np.diagonal(np.asarray(W_h, np.float32)))
    dcols = np.ascontiguousarray(d.reshape(NH, 128).T, dtype=np.float32)
    in_maps = []
    for c in range(NCORES):
        in_maps.append({
            "xT": np.ascontiguousarray(xT[c * BPC:(c + 1) * BPC]),
            "w_inT": w_inT,
            "w_outT": w_outT,
            "dcols": dcols,
        })
    return in_maps


def _get_nc(mode: str = MODE_DEFAULT):
    key = (mode, tuple(sorted(CFG.items())))
    if key not in _cache:
        _cache[key] = _build(mode)
    return _cache[key]


def _run(x, W_in, W_h, W_out, mode: str = MODE_DEFAULT, **spmd_kwargs):
    nc = _get_nc(mode)
    in_maps = _prep_inputs(x, W_in, W_h, W_out, mode)
    res = run_bass_kernel_spmd(nc, in_maps, list(range(NCORES)), **spmd_kwargs)
    parts = [np.transpose(np.asarray(res.results[c]["out"]), (0, 2, 1))
             for c in range(NCORES)]
    full = np.concatenate(parts, axis=0).astype(np.float32)
    return full, res


def kernel(x, W_in, W_h, W_out):
    out, _ = _run(x, W_in, W_h, W_out)
    return out



# revision 2
# speedup vs baseline: 1.0791x; 1.0791x over previous
"""Trainium2 Bass kernel for nn_LinearRNN (B=16, T=4096, D_in=256, H=512, D_out=256).

  xp = x @ W_in.T                       [B, T, H]
  h_t = xp_t + h_{t-1} @ W_h.T          (W_h is diagonal -> elementwise scan)
  out = hs @ W_out.T                    [B, T, D_out]

Strategy: batch data-parallel over 8 cores (2 batch rows per core). Per core:
  - host pre-transposes x to [b, d, t] so the contraction dim lands on SBUF
    partitions; weights pre-transposed likewise.
  - matmul1 on TensorE produces xp tiles [h=128, t=512] in PSUM. In fp8 mode
    (default) it runs as 3 DoubleRow fp8 passes (0.5 cyc/row, both d-blocks
    per pass): W_hi@x_hi + W_hi@x_lo + W_lo@x_hi, where x and W_in are
    residual-split into e4m3 hi+lo pairs ON THE HOST with scales (8, 64)
    chosen to keep the lo parts out of the fp8 subnormal range. The dropped
    W_lo@x_lo term is O(eps^2). The 1/512 scale folds into W_out.
  - VectorE tensor_tensor_scan runs the recurrence along the free (t) axis
    with the per-h decay broadcast from a [128,1] column, carry chained
    across t-chunks via the previous tile's last column (hs stays f32r:
    chunk-boundary rounding of the carry must stay well above bf16),
  - matmul2 on TensorE contracts h back to d_out in f32r, ScalarE copies
    PSUM->SBUF, out [b, o, t] DMAs back, host transposes to [b, t, o].
"""
from contextlib import ExitStack

import numpy as np

import concourse.bass as bass
import concourse.mybir as mybir
import concourse.tile as tile
from concourse import bacc
from concourse.bass_utils import run_bass_kernel_spmd

B, T, D_IN, HID, D_OUT = 16, 4096, 256, 512, 256
NCORES = 8
BPC = B // NCORES          # batch rows per core
TC = 512                   # t-chunk (PSUM bank = 512 fp32)
NCH = T // TC
ND = D_IN // 128           # 2  d-blocks
NH = HID // 128            # 4  h-blocks
NO = D_OUT // 128          # 2  o-blocks

SX = 8.0                   # host scale on x before fp8 split
SW = 64.0                  # host scale on W_in before fp8 split

# 'fp8'  : mm1 as 3 fp8-e4m3 DoubleRow passes (hi/lo residual split), mm2 f32r
# 'f32r' : fp32 storage, PE runs reduced-precision single-pass (1 cyc/row)
MODE_DEFAULT = "fp8"

# schedule/tuning knobs (read by _build; cache key includes them)
CFG = dict(sched="pipe1", xp_bufs=4, op_bufs=4, hs_bufs=16,
           x_piece=1024, out_piece=512)

_cache: dict = {}


def _build_fp8() -> bass.Bass:
    f32 = mybir.dt.float32
    f32r = mybir.dt.float32r
    f8 = mybir.dt.float8e4
    DR = mybir.MatmulPerfMode.DoubleRow

    nc = bacc.Bacc(None, target_bir_lowering=False)

    xh8 = nc.declare_dram_parameter("xh8", [BPC, ND, 128, T], f8, isOutput=False)
    xl8 = nc.declare_dram_parameter("xl8", [BPC, ND, 128, T], f8, isOutput=False)
    wih = nc.declare_dram_parameter("wih", [ND, 128, HID], f8, isOutput=False)
    wil = nc.declare_dram_parameter("wil", [ND, 128, HID], f8, isOutput=False)
    w_outT = nc.declare_dram_parameter("w_outT", [HID, D_OUT], f32r, isOutput=False)
    dcols = nc.declare_dram_parameter("dcols", [128, NH], f32, isOutput=False)
    out = nc.declare_dram_parameter("out", [BPC, D_OUT, T], f32, isOutput=True)

    with tile.TileContext(nc) as tc, ExitStack() as ctx:
        const_pool = ctx.enter_context(tc.tile_pool(name="const", bufs=1))
        x_pool = ctx.enter_context(tc.tile_pool(name="xt", bufs=2 * BPC))
        o_pool = ctx.enter_context(tc.tile_pool(name="ot", bufs=8))
        hs_pool = ctx.enter_context(tc.tile_pool(name="hs", bufs=CFG["hs_bufs"]))
        xp_psum = ctx.enter_context(
            tc.tile_pool(name="xp", bufs=CFG["xp_bufs"], space=bass.MemorySpace.PSUM))
        op_psum = ctx.enter_context(
            tc.tile_pool(name="op", bufs=CFG["op_bufs"], space=bass.MemorySpace.PSUM))

        XP_LEN = CFG["x_piece"]
        xth, xtl = {}, {}
        for b in range(BPC):
            xth[b] = x_pool.tile([128, ND, T], f8, name="xth", tag="xth")
            xtl[b] = x_pool.tile([128, ND, T], f8, name="xtl", tag="xtl")

        def load_x(b, piece, lo):
            psl = slice(piece * XP_LEN, (piece + 1) * XP_LEN)
            dst, src = (xtl[b], xl8) if lo else (xth[b], xh8)
            for j in range(ND):
                nc.sync.dma_start(dst[:, j, psl], src[b, j, :, psl])

        # DMA emission order is dispatch order per queue: matmul1 pass-1
        # operands first (wih + x_hi piece 0) so PE unblocks early, then the
        # pass-2/3 operands, remaining constants, and the rest of x.
        wih_sb = const_pool.tile([128, ND, HID], f8, tag="wih")
        for j in range(ND):
            nc.sync.dma_start(wih_sb[:, j, :], wih[j])
        load_x(0, 0, lo=False)
        load_x(0, 0, lo=True)
        wil_sb = const_pool.tile([128, ND, HID], f8, tag="wil")
        for j in range(ND):
            nc.sync.dma_start(wil_sb[:, j, :], wil[j])
        dc = const_pool.tile([128, NH], f32, tag="dc")
        nc.sync.dma_start(dc[:], dcols[:])
        wo = []
        for hblk in range(NH):
            w = const_pool.tile([128, D_OUT], f32r, tag=f"wo{hblk}")
            nc.sync.dma_start(w[:], w_outT[hblk * 128:(hblk + 1) * 128, :])
            wo.append(w)
        for piece in range(1, T // XP_LEN):
            load_x(0, piece, lo=False)
            load_x(0, piece, lo=True)
        for b in range(1, BPC):
            for piece in range(T // XP_LEN):
                load_x(b, piece, lo=False)
                load_x(b, piece, lo=True)

        OP = CFG["out_piece"]
        ot = {}  # (b, oblk) -> current staging piece, created lazily

        prev_hs = {}

        def stage1(b, ic):
            """matmul1 (3 fp8 DoubleRow passes) + scan per h-block."""
            tsl = slice(ic * TC, (ic + 1) * TC)
            for hblk in range(NH):
                hsl = slice(hblk * 128, (hblk + 1) * 128)
                xp = xp_psum.tile([128, TC], f32, name="xp", tag="xp")
                nc.tensor.matmul(xp[:], wih_sb[:, :, hsl], xth[b][:, :, tsl],
                                 start=True, stop=False, perf_mode=DR)
                nc.tensor.matmul(xp[:], wih_sb[:, :, hsl], xtl[b][:, :, tsl],
                                 start=False, stop=False, perf_mode=DR)
                nc.tensor.matmul(xp[:], wil_sb[:, :, hsl], xth[b][:, :, tsl],
                                 start=False, stop=True, perf_mode=DR)
                hs = hs_pool.tile([128, TC], f32r, name="hs", tag="hs")
                init = (0.0 if ic == 0
                        else prev_hs[(b, ic - 1, hblk)][:, TC - 1:TC])
                nc.vector.tensor_tensor_scan(
                    hs[:], dc[:, hblk:hblk + 1].to_broadcast((128, TC)),
                    xp[:], init,
                    op0=mybir.AluOpType.mult, op1=mybir.AluOpType.add)
                prev_hs[(b, ic, hblk)] = hs

        def stage2(b, ic):
            """matmul2 (f32r) + PSUM->SBUF copy (+ out DMA) per o-block."""
            q, csl = divmod(ic * TC, OP)
            for oblk in range(NO):
                op = op_psum.tile([128, TC], f32, name="op", tag="op")
                for hblk in range(NH):
                    nc.tensor.matmul(
                        op[:],
                        wo[hblk][:, oblk * 128:(oblk + 1) * 128],
                        prev_hs[(b, ic, hblk)][:],
                        start=(hblk == 0), stop=(hblk == NH - 1))
                if csl == 0:
                    ot[(b, oblk)] = o_pool.tile([128, OP], f32,
                                                name="ot", tag="ot")
                nc.scalar.copy(ot[(b, oblk)][:, csl:csl + TC], op[:])
                if csl + TC == OP:
                    nc.sync.dma_start(
                        out[b, oblk * 128:(oblk + 1) * 128,
                            q * OP:(q + 1) * OP],
                        ot[(b, oblk)][:])

        sched = CFG.get("sched", "pipe1")
        if sched == "pipe1":
            # software pipeline: keep next chunk's matmul1s ahead of the
            # scan-dependent matmul2s in PE program order
            for b in range(BPC):
                stage1(b, 0)
                for ic in range(NCH - 1):
                    stage1(b, ic + 1)
                    stage2(b, ic)
                stage2(b, NCH - 1)
        elif sched == "pipe1x":
            # as pipe1, but cross-batch: b1 chunk 0 primes before b0 drains
            order = [(b, ic) for b in range(BPC) for ic in range(NCH)]
            stage1(*order[0])
            for k in range(len(order) - 1):
                stage1(*order[k + 1])
                stage2(*order[k])
            stage2(*order[-1])
        else:
            for b in range(BPC):
                for ic in range(NCH):
                    stage1(b, ic)
                    stage2(b, ic)

    nc.compile()
    return nc


def _build_f32r() -> bass.Bass:
    """Baseline f32r variant (kept as fallback)."""
    f32 = mybir.dt.float32
    dt_in = mybir.dt.float32r
    dt_hs = dt_in

    nc = bacc.Bacc(None, target_bir_lowering=False)

    xT = nc.declare_dram_parameter("xT", [BPC, D_IN, T], dt_in, isOutput=False)
    w_inT = nc.declare_dram_parameter("w_inT", [D_IN, HID], dt_in, isOutput=False)
    w_outT = nc.declare_dram_parameter("w_outT", [HID, D_OUT], dt_in, isOutput=False)
    dcols = nc.declare_dram_parameter("dcols", [128, NH], f32, isOutput=False)
    out = nc.declare_dram_parameter("out", [BPC, D_OUT, T], f32, isOutput=True)

    with tile.TileContext(nc) as tc, ExitStack() as ctx:
        const_pool = ctx.enter_context(tc.tile_pool(name="const", bufs=1))
        x_pool = ctx.enter_context(tc.tile_pool(name="xt", bufs=BPC * ND))
        o_pool = ctx.enter_context(tc.tile_pool(name="ot", bufs=8))
        hs_pool = ctx.enter_context(tc.tile_pool(name="hs", bufs=CFG["hs_bufs"]))
        xp_psum = ctx.enter_context(
            tc.tile_pool(name="xp", bufs=CFG["xp_bufs"], space=bass.MemorySpace.PSUM))
        op_psum = ctx.enter_context(
            tc.tile_pool(name="op", bufs=CFG["op_bufs"], space=bass.MemorySpace.PSUM))

        XP_LEN = 512
        xt = {}
        for b in range(BPC):
            for dblk in range(ND):
                xt[(b, dblk)] = x_pool.tile([128, T], dt_in, name="xt", tag="xt")

        def load_x(b, dblk, piece):
            psl = slice(piece * XP_LEN, (piece + 1) * XP_LEN)
            nc.sync.dma_start(xt[(b, dblk)][:, psl],
                              xT[b, dblk * 128:(dblk + 1) * 128, psl])

        for dblk in range(ND):
            load_x(0, dblk, 0)
        wi = []
        for dblk in range(ND):
            w = const_pool.tile([128, HID], dt_in, tag=f"wi{dblk}")
            nc.sync.dma_start(w[:], w_inT[dblk * 128:(dblk + 1) * 128, :])
            wi.append(w)
        wo = []
        for hblk in range(NH):
            w = const_pool.tile([128, D_OUT], dt_in, tag=f"wo{hblk}")
            nc.sync.dma_start(w[:], w_outT[hblk * 128:(hblk + 1) * 128, :])
            wo.append(w)
        dc = const_pool.tile([128, NH], f32, tag="dc")
        nc.sync.dma_start(dc[:], dcols[:])
        for piece in range(1, T // XP_LEN):
            for dblk in range(ND):
                load_x(0, dblk, piece)
        for b in range(1, BPC):
            for piece in range(T // XP_LEN):
                for dblk in range(ND):
                    load_x(b, dblk, piece)

        OP = CFG["out_piece"]
        ot = {}
        prev_hs = {}

        def stage1(b, ic):
            tsl = slice(ic * TC, (ic + 1) * TC)
            for hblk in range(NH):
                xp = xp_psum.tile([128, TC], f32, name="xp", tag="xp")
                for dblk in range(ND):
                    nc.tensor.matmul(
                        xp[:],
                        wi[dblk][:, hblk * 128:(hblk + 1) * 128],
                        xt[(b, dblk)][:, tsl],
                        start=(dblk == 0), stop=(dblk == ND - 1))
                hs = hs_pool.tile([128, TC], dt_hs, name="hs", tag="hs")
                init = (0.0 if ic == 0
                        else prev_hs[(b, ic - 1, hblk)][:, TC - 1:TC])
                nc.vector.tensor_tensor_scan(
                    hs[:], dc[:, hblk:hblk + 1].to_broadcast((128, TC)),
                    xp[:], init,
                    op0=mybir.AluOpType.mult, op1=mybir.AluOpType.add)
                prev_hs[(b, ic, hblk)] = hs

        def stage2(b, ic):
            q, csl = divmod(ic * TC, OP)
            for oblk in range(NO):
                op = op_psum.tile([128, TC], f32, name="op", tag="op")
                for hblk in range(NH):
                    nc.tensor.matmul(
                        op[:],
                        wo[hblk][:, oblk * 128:(oblk + 1) * 128],
                        prev_hs[(b, ic, hblk)][:],
                        start=(hblk == 0), stop=(hblk == NH - 1))
                if csl == 0:
                    ot[(b, oblk)] = o_pool.tile([128, OP], f32,
                                                name="ot", tag="ot")
                nc.scalar.copy(ot[(b, oblk)][:, csl:csl + TC], op[:])
                if csl + TC == OP:
                    nc.sync.dma_start(
                        out[b, oblk * 128:(oblk + 1) * 128,
                            q * OP:(q + 1) * OP],
                        ot[(b, oblk)][:])

        for b in range(BPC):
            stage1(b, 0)
            for ic in range(NCH - 1):
                stage1(b, ic + 1)
                stage2(b, ic)
            stage2(b, NCH - 1)

    nc.compile()
    return nc


def _build(mode: str) -> bass.Bass:
    if mode == "fp8":
        return _build_fp8()
    return _build_f32r()


def _prep_inputs(x, W_in, W_h, W_out, mode: str):
    import ml_dtypes
    f8 = ml_dtypes.float8_e4m3
    d = np.ascontiguousarray(np.diagonal(np.asarray(W_h, np.float32)))
    dcols = np.ascontiguousarray(d.reshape(NH, 128).T, dtype=np.float32)

    if mode == "fp8":
        xs = np.transpose(np.asarray(x, np.float32), (0, 2, 1)) * SX  # [B,D,T]
        xh = xs.astype(f8)
        xl = (xs - xh.astype(np.float32)).astype(f8)
        xh = np.ascontiguousarray(xh.reshape(B, ND, 128, T))
        xl = np.ascontiguousarray(xl.reshape(B, ND, 128, T))
        ws = np.asarray(W_in, np.float32).T * SW                      # [D,H]
        wh = ws.astype(f8)
        wl = (ws - wh.astype(np.float32)).astype(f8)
        wih = np.ascontiguousarray(wh.reshape(ND, 128, HID))
        wil = np.ascontiguousarray(wl.reshape(ND, 128, HID))
        w_outT = np.ascontiguousarray(
            np.asarray(W_out, np.float32).T / (SX * SW))
        in_maps = []
        for c in range(NCORES):
            in_maps.append({
                "xh8": np.ascontiguousarray(xh[c * BPC:(c + 1) * BPC]),
                "xl8": np.ascontiguousarray(xl[c * BPC:(c + 1) * BPC]),
                "wih": wih,
                "wil": wil,
                "w_outT": w_outT,
                "dcols": dcols,
            })
        return in_maps

    xT = np.ascontiguousarray(
        np.transpose(np.asarray(x, np.float32), (0, 2, 1)))
    w_inT = np.ascontiguousarray(np.asarray(W_in, np.float32).T)
    w_outT = np.ascontiguousarray(np.asarray(W_out, np.float32).T)
    in_maps = []
    for c in range(NCORES):
        in_maps.append({
            "xT": np.ascontiguousarray(xT[c * BPC:(c + 1) * BPC]),
            "w_inT": w_inT,
            "w_outT": w_outT,
            "dcols": dcols,
        })
    return in_maps


def _get_nc(mode: str = MODE_DEFAULT):
    key = (mode, tuple(sorted(CFG.items())))
    if key not in _cache:
        _cache[key] = _build(mode)
    return _cache[key]


def _run(x, W_in, W_h, W_out, mode: str = MODE_DEFAULT, **spmd_kwargs):
    nc = _get_nc(mode)
    in_maps = _prep_inputs(x, W_in, W_h, W_out, mode)
    res = run_bass_kernel_spmd(nc, in_maps, list(range(NCORES)), **spmd_kwargs)
    parts = [np.transpose(np.asarray(res.results[c]["out"]), (0, 2, 1))
             for c in range(NCORES)]
    full = np.concatenate(parts, axis=0).astype(np.float32)
    return full, res


def kernel(x, W_in, W_h, W_out):
    out, _ = _run(x, W_in, W_h, W_out)
    return out
